# revision 15
# baseline (speedup 1.0000x reference)
"""Trainium2 Bass kernel for nn_MultiHeadGraphAttentionLayer_28956669509709.

Mathematical reduction (verified numerically, norm rel err ~2e-7 vs the
reference):

  The reference computes att = softmax(e, axis=-1) and then uses it only as
  sum(att, axis=-1, keepdims=True), which is identically ~1.0 (a softmax row
  sums to one).  Hence out = hp * 1 = hp: the whole [H,B,N,N] attention
  tensor is dead, and `adj` is unused (drop_edge is identity).

  With a = clip(res_alpha, 0, 1) and Wc = concat_heads(W) [F_in, H*D]:
      out_b = a * (h_b @ Wc) + (1 - a) * h_b = h_b @ (a * Wc + (1 - a) * I)

  so the problem collapses to one [2048,256] @ [256,256] matmul per batch
  sample plus nothing else.  Sharding: data-parallel over B=8 across the 8
  NeuronCores (one sample per core), replicating the fused weight
  M = a*Wc + (1-a)*I.

Numerics: the matmul runs as a bf16 hi/lo split (3 passes, fp32 PSUM
accumulate).  h = hh + hl and M = mh + ml in bf16; bf16 products are exact
in fp32, so  hh@mh + hh@ml + hl@mh  reproduces the fp32 product up to the
dropped hl@ml term (~2^-18 relative).  Measured end-to-end rel err vs the
fp32 reference: ~5e-6.  Each pass streams at 1 cycle/row on the PE (vs 4
for fp32), and input traffic equals fp32 (two bf16 halves = 4 B/elem).

Layout: the TensorEngine contracts along the partition dim for both
operands, so h is fed f-major (pre-transposed on host, cheap).  The kernel
needs zero on-chip transposes and writes the output in natural layout.

Schedule (raw Bass, no Tile framework -> no kernel-tail drain/barrier
butterfly):
  SP   : load m (2 DMAs), load h in 8 column chunks x 2 k-halves (16 DMAs,
         one [128, 512]-bf16 tile per DMA holding [hh | hl]).
  PE   : per PSUM bank (2 token tiles, never reused -> 8 banks): 12
         accumulating matmuls (2 tiles x 3 passes x 2 k-chunks).
  DVE  : copy each finished PSUM bank into an SBUF out tile.
  ACT  : 8 store DMAs on the second HWDGE ring (parallel with SP's loads),
         final completion wait.
"""

from contextlib import ExitStack

import ml_dtypes
import numpy as np

from concourse import bacc, mybir
from concourse.bass_utils import run_bass_kernel_spmd

F32 = mybir.dt.float32
BF16 = mybir.dt.bfloat16

# Problem geometry (fixed by the problem spec).
B, N, F_IN = 8, 2048, 256
H, D = 4, 64
FO = H * D              # 256 output features
N_CORES = 8

P = 128                 # SBUF partitions
KC = F_IN // P          # contraction chunks (2)
TT = N // P             # token tiles (16)
N_CHUNKS = 8            # h column chunks per k-half
N_STORES = 8            # store DMAs
TPC = TT // N_CHUNKS    # token tiles per chunk (2)
CW = TPC * P            # tokens per chunk (256)
TILES_PER_STORE = TT // N_STORES   # 2
STORE_COLS = TILES_PER_STORE * FO  # 512

_NC_CACHE = {}


def _build_nc():
    nc = bacc.Bacc("TRN2", target_bir_lowering=False, debug=False)
    # x row f = k*128 + p; per chunk c, columns [c*2cw, (c+1)*2cw) hold
    # [hh chunk | hl chunk] (bf16 hi/lo split of hT).
    x_d = nc.dram_tensor("x", [F_IN, 2 * N], BF16, kind="ExternalInput")
    # m row f: [mh | ml]
    m_d = nc.dram_tensor("m", [F_IN, 2 * FO], BF16, kind="ExternalInput")
    out_d = nc.dram_tensor("out", [N, FO], F32, kind="ExternalOutput")
    out_v = out_d.rearrange("(v q p) d -> v p q d", q=TILES_PER_STORE, p=P)

    with ExitStack() as ctx:
        m_sb = [
            ctx.enter_context(nc.sbuf_tensor(f"m{k}", [P, 2 * FO], BF16))
            for k in range(KC)
        ]
        h_sb = [
            [
                ctx.enter_context(
                    nc.sbuf_tensor(f"h{k}_{c}", [P, 2 * CW], BF16)
                )
                for c in range(N_CHUNKS)
            ]
            for k in range(KC)
        ]
        o_sb = [
            ctx.enter_context(nc.sbuf_tensor(f"o{v}", [P, STORE_COLS], F32))
            for v in range(N_STORES)
        ]
        ps_banks = [
            ctx.enter_context(nc.psum_tensor(f"ps{g}", [P, 2 * FO], F32))
            for g in range(8)
        ]
        s_m = ctx.enter_context(nc.semaphore("s_m"))
        s_h = [
            ctx.enter_context(nc.semaphore(f"s_h{c}")) for c in range(N_CHUNKS)
        ]
        pe_sem = ctx.enter_context(nc.semaphore("pe_sem"))
        dve_sem = ctx.enter_context(nc.semaphore("dve_sem"))
        out_sem = ctx.enter_context(nc.semaphore("out_sem"))
        block = ctx.enter_context(nc.Block())

        banks_per_store = TILES_PER_STORE // 2  # 1

        @block.sync
        def _(sync):
            for k in range(KC):
                sync.dma_start(
                    m_sb[k][:], m_d[k * P : (k + 1) * P, :]
                ).then_inc(s_m, 16)
            for c in range(N_CHUNKS):
                for k in range(KC):
                    sync.dma_start(
                        h_sb[k][c][:],
                        x_d[k * P : (k + 1) * P, c * 2 * CW : (c + 1) * 2 * CW],
                    ).then_inc(s_h[c], 16)

        @block.tensor
        def _(tensor):
            tensor.wait_ge(s_m, 16 * KC)
            seen = set()
            for g in range(8):
                for s in range(2):
                    t = 2 * g + s
                    c, i = divmod(t, TPC)
                    if c not in seen:
                        tensor.wait_ge(s_h[c], 16 * KC)
                        seen.add(c)
                    # (h part, m part) passes; hl@ml dropped (~2^-18).
                    passes = [(0, 0), (0, 1), (1, 0)]
                    n_mm = len(passes) * KC
                    done = 0
                    for hp, mp in passes:
                        for k in range(KC):
                            mm = nc.tensor.matmul(
                                ps_banks[g][:, s * FO : (s + 1) * FO],
                                h_sb[k][c][
                                    :, hp * CW + i * P : hp * CW + (i + 1) * P
                                ],
                                m_sb[k][:, mp * FO : (mp + 1) * FO],
                                start=(done == 0),
                                stop=(done == n_mm - 1),
                            )
                            done += 1
                            if s == 1 and done == n_mm:
                                mm.then_inc(pe_sem, 1)

        @block.vector
        def _(vector):
            for g in range(8):
                vector.wait_ge(pe_sem, g + 1)
                t0 = 2 * g
                v, off = divmod(t0 * FO, STORE_COLS)
                nc.vector.tensor_copy(
                    o_sb[v][:, off : off + 2 * FO], ps_banks[g][:]
                ).then_inc(dve_sem, 1)

        @block.scalar
        def _(scalar):
            for v in range(N_STORES):
                scalar.wait_ge(dve_sem, (v + 1) * banks_per_store)
                scalar.dma_start(
                    out_v[v],
                    o_sb[v].rearrange("p (q d) -> p q d", q=TILES_PER_STORE),
                ).then_inc(out_sem, 16)
            scalar.wait_ge(out_sem, 16 * N_STORES)

    nc.compile()
    return nc


def _get_nc():
    if "nc" not in _NC_CACHE:
        _NC_CACHE["nc"] = _build_nc()
    return _NC_CACHE["nc"]


def _pack_inputs(h, W, res_alpha):
    bf16 = ml_dtypes.bfloat16
    h = np.asarray(h, dtype=np.float32)
    W = np.asarray(W, dtype=np.float32)

    a = float(np.clip(np.float32(res_alpha), 0.0, 1.0))
    # Concat heads: f_out = head*D + d  ->  Wc[f_in, f_out]
    Wc = np.transpose(W, (1, 0, 2)).reshape(F_IN, FO)
    M = (a * Wc + (1.0 - a) * np.eye(F_IN, dtype=np.float32)).astype(np.float32)
    mh = M.astype(bf16)
    ml = (M - mh.astype(np.float32)).astype(bf16)
    m_packed = np.ascontiguousarray(np.concatenate([mh, ml], axis=1))

    in_maps = []
    for b in range(B):
        hT = np.ascontiguousarray(h[b].T)  # [F_IN, N]
        hh = hT.astype(bf16)
        hl = (hT - hh.astype(np.float32)).astype(bf16)
        x = np.empty((F_IN, 2 * N), dtype=bf16)
        for c in range(N_CHUNKS):
            base = c * 2 * CW
            x[:, base : base + CW] = hh[:, c * CW : (c + 1) * CW]
            x[:, base + CW : base + 2 * CW] = hl[:, c * CW : (c + 1) * CW]
        in_maps.append({"x": x, "m": m_packed})
    return in_maps


def kernel(h, adj, W, res_alpha, **_unused):
    h = np.asarray(h, dtype=np.float32)
    W = np.asarray(W, dtype=np.float32)
    assert h.shape == (B, N, F_IN), h.shape
    assert W.shape == (H, F_IN, D), W.shape

    in_maps = _pack_inputs(h, W, res_alpha)
    res = run_bass_kernel_spmd(_get_nc(), in_maps, list(range(N_CORES)))
    out = np.stack([res.results[b]["out"] for b in range(B)], axis=0)
    return np.ascontiguousarray(out.astype(np.float32))


# revision 16
# speedup vs baseline: 1.1097x; 1.1097x over previous
"""Trainium2 Bass kernel for nn_MultiHeadGraphAttentionLayer_28956669509709.

Mathematical reduction (verified numerically, norm rel err ~2e-7 vs the
reference):

  The reference computes att = softmax(e, axis=-1) and then uses it only as
  sum(att, axis=-1, keepdims=True), which is identically ~1.0 (a softmax row
  sums to one).  Hence out = hp * 1 = hp: the whole [H,B,N,N] attention
  tensor is dead, and `adj` is unused (drop_edge is identity).

  With a = clip(res_alpha, 0, 1) and Wc = concat_heads(W) [F_in, H*D]:
      out_b = a * (h_b @ Wc) + (1 - a) * h_b = h_b @ (a * Wc + (1 - a) * I)

  so the problem collapses to one [2048,256] @ [256,256] matmul per batch
  sample plus nothing else.  Sharding: data-parallel over B=8 across the 8
  NeuronCores (one sample per core), replicating the fused weight
  M = a*Wc + (1-a)*I.

Numerics: the matmul runs as a bf16 hi/lo split (3 passes, fp32 PSUM
accumulate).  h = hh + hl and M = mh + ml in bf16; bf16 products are exact
in fp32, so  hh@mh + hh@ml + hl@mh  reproduces the fp32 product up to the
dropped hl@ml term (~2^-18 relative).  Measured end-to-end rel err vs the
fp32 reference: ~5e-6.  Each pass streams at 1 cycle/row on the PE (vs 4
for fp32), and input traffic equals fp32 (two bf16 halves = 4 B/elem).

Layout: the TensorEngine contracts along the partition dim for both
operands, so h is fed f-major (pre-transposed on host, cheap).  The kernel
needs zero on-chip transposes and writes the output in natural layout.

Schedule (raw Bass, no Tile framework -> no kernel-tail drain/barrier
butterfly).  DMA count is deliberately small: the HWDGE is a serially
shared resource (~0.6us/DMA) and extra load DMAs starve the store stream,
which otherwise backlogs into a multi-microsecond tail after the PE
finishes (observed in the timeline sim):
  SP   : 8 load DMAs, one per column chunk -- each fills BOTH k-halves of
         a [128, 1024]-bf16 tile ([k0: hh|hl, k1: hh|hl]) via a 3-D access
         pattern.
  ACT  : 1 DMA for m (both k-halves, [mh|ml] each), then 8 store DMAs on
         the second HWDGE ring, final completion wait.
  PE   : per PSUM bank (2 token tiles, never reused -> 8 banks): 12
         accumulating matmuls (2 tiles x 3 passes x 2 k-chunks).
  DVE  : copy each finished PSUM bank into an SBUF out tile.
"""

from contextlib import ExitStack

import ml_dtypes
import numpy as np

from concourse import bacc, mybir
from concourse.bass_utils import run_bass_kernel_spmd

F32 = mybir.dt.float32
BF16 = mybir.dt.bfloat16

# Problem geometry (fixed by the problem spec).
B, N, F_IN = 8, 2048, 256
H, D = 4, 64
FO = H * D              # 256 output features
N_CORES = 8

P = 128                 # SBUF partitions
KC = F_IN // P          # contraction chunks (2)
TT = N // P             # token tiles (16)
N_CHUNKS = 8            # h column chunks per k-half
N_STORES = 8            # store DMAs
TPC = TT // N_CHUNKS    # token tiles per chunk (2)
CW = TPC * P            # tokens per chunk (256)
TILES_PER_STORE = TT // N_STORES   # 2
STORE_COLS = TILES_PER_STORE * FO  # 512

_NC_CACHE = {}


def _build_nc():
    nc = bacc.Bacc("TRN2", target_bir_lowering=False, debug=False)
    # x row f = k*128 + p; per chunk c, columns [c*2cw, (c+1)*2cw) hold
    # [hh chunk | hl chunk] (bf16 hi/lo split of hT).
    x_d = nc.dram_tensor("x", [F_IN, 2 * N], BF16, kind="ExternalInput")
    # m row f: [mh | ml]
    m_d = nc.dram_tensor("m", [F_IN, 2 * FO], BF16, kind="ExternalInput")
    out_d = nc.dram_tensor("out", [N, FO], F32, kind="ExternalOutput")
    out_v = out_d.rearrange("(v q p) d -> v p q d", q=TILES_PER_STORE, p=P)
    x_v = x_d.rearrange("(k p) w -> p k w", k=KC)
    m_v = m_d.rearrange("(k p) w -> p k w", k=KC)

    with ExitStack() as ctx:
        m_sb = ctx.enter_context(
            nc.sbuf_tensor("m01", [P, KC * 2 * FO], BF16)
        )
        h_sb = [
            ctx.enter_context(
                nc.sbuf_tensor(f"h{c}", [P, KC * 2 * CW], BF16)
            )
            for c in range(N_CHUNKS)
        ]
        o_sb = [
            ctx.enter_context(nc.sbuf_tensor(f"o{v}", [P, STORE_COLS], F32))
            for v in range(N_STORES)
        ]
        ps_banks = [
            ctx.enter_context(nc.psum_tensor(f"ps{g}", [P, 2 * FO], F32))
            for g in range(8)
        ]
        s_m = ctx.enter_context(nc.semaphore("s_m"))
        s_h = [
            ctx.enter_context(nc.semaphore(f"s_h{c}")) for c in range(N_CHUNKS)
        ]
        pe_sem = ctx.enter_context(nc.semaphore("pe_sem"))
        dve_sem = ctx.enter_context(nc.semaphore("dve_sem"))
        out_sem = ctx.enter_context(nc.semaphore("out_sem"))
        block = ctx.enter_context(nc.Block())

        banks_per_store = TILES_PER_STORE // 2  # 1

        @block.sync
        def _(sync):
            for c in range(N_CHUNKS):
                sync.dma_start(
                    h_sb[c].rearrange("p (k w) -> p k w", k=KC),
                    x_v[:, :, c * 2 * CW : (c + 1) * 2 * CW],
                ).then_inc(s_h[c], 16)

        @block.tensor
        def _(tensor):
            tensor.wait_ge(s_m, 16)
            seen = set()
            for g in range(8):
                for s in range(2):
                    t = 2 * g + s
                    c, i = divmod(t, TPC)
                    if c not in seen:
                        tensor.wait_ge(s_h[c], 16)
                        seen.add(c)
                    # (h part, m part) passes; hl@ml dropped (~2^-18).
                    passes = [(0, 0), (0, 1), (1, 0)]
                    n_mm = len(passes) * KC
                    done = 0
                    for hp, mp in passes:
                        for k in range(KC):
                            mm = nc.tensor.matmul(
                                ps_banks[g][:, s * FO : (s + 1) * FO],
                                h_sb[c][
                                    :,
                                    k * 2 * CW + hp * CW + i * P
                                    : k * 2 * CW + hp * CW + (i + 1) * P,
                                ],
                                m_sb[:, k * 2 * FO + mp * FO : k * 2 * FO + (mp + 1) * FO],
                                start=(done == 0),
                                stop=(done == n_mm - 1),
                            )
                            done += 1
                            if s == 1 and done == n_mm:
                                mm.then_inc(pe_sem, 1)

        @block.vector
        def _(vector):
            for g in range(8):
                vector.wait_ge(pe_sem, g + 1)
                t0 = 2 * g
                v, off = divmod(t0 * FO, STORE_COLS)
                nc.vector.tensor_copy(
                    o_sb[v][:, off : off + 2 * FO], ps_banks[g][:]
                ).then_inc(dve_sem, 1)

        @block.scalar
        def _(scalar):
            scalar.dma_start(
                m_sb.rearrange("p (k w) -> p k w", k=KC), m_v[:]
            ).then_inc(s_m, 16)
            for v in range(N_STORES):
                scalar.wait_ge(dve_sem, (v + 1) * banks_per_store)
                scalar.dma_start(
                    out_v[v],
                    o_sb[v].rearrange("p (q d) -> p q d", q=TILES_PER_STORE),
                ).then_inc(out_sem, 16)
            scalar.wait_ge(out_sem, 16 * N_STORES)

    nc.compile()
    return nc


def _get_nc():
    if "nc" not in _NC_CACHE:
        _NC_CACHE["nc"] = _build_nc()
    return _NC_CACHE["nc"]


def _pack_inputs(h, W, res_alpha):
    bf16 = ml_dtypes.bfloat16
    h = np.asarray(h, dtype=np.float32)
    W = np.asarray(W, dtype=np.float32)

    a = float(np.clip(np.float32(res_alpha), 0.0, 1.0))
    # Concat heads: f_out = head*D + d  ->  Wc[f_in, f_out]
    Wc = np.transpose(W, (1, 0, 2)).reshape(F_IN, FO)
    M = (a * Wc + (1.0 - a) * np.eye(F_IN, dtype=np.float32)).astype(np.float32)
    mh = M.astype(bf16)
    ml = (M - mh.astype(np.float32)).astype(bf16)
    m_packed = np.ascontiguousarray(np.concatenate([mh, ml], axis=1))

    in_maps = []
    for b in range(B):
        hT = np.ascontiguousarray(h[b].T)  # [F_IN, N]
        hh = hT.astype(bf16)
        hl = (hT - hh.astype(np.float32)).astype(bf16)
        x = np.empty((F_IN, 2 * N), dtype=bf16)
        for c in range(N_CHUNKS):
            base = c * 2 * CW
            x[:, base : base + CW] = hh[:, c * CW : (c + 1) * CW]
            x[:, base + CW : base + 2 * CW] = hl[:, c * CW : (c + 1) * CW]
        in_maps.append({"x": x, "m": m_packed})
    return in_maps


def kernel(h, adj, W, res_alpha, **_unused):
    h = np.asarray(h, dtype=np.float32)
    W = np.asarray(W, dtype=np.float32)
    assert h.shape == (B, N, F_IN), h.shape
    assert W.shape == (H, F_IN, D), W.shape

    in_maps = _pack_inputs(h, W, res_alpha)
    res = run_bass_kernel_spmd(_get_nc(), in_maps, list(range(N_CORES)))
    out = np.stack([res.results[b]["out"] for b in range(B)], axis=0)
    return np.ascontiguousarray(out.astype(np.float32))


# revision 17
# speedup vs baseline: 1.2073x; 1.0879x over previous
"""Trainium2 Bass kernel for nn_MultiHeadGraphAttentionLayer_28956669509709.

Mathematical reduction (verified numerically, norm rel err ~2e-7 vs the
reference):

  The reference computes att = softmax(e, axis=-1) and then uses it only as
  sum(att, axis=-1, keepdims=True), which is identically ~1.0 (a softmax row
  sums to one).  Hence out = hp * 1 = hp: the whole [H,B,N,N] attention
  tensor is dead, and `adj` is unused (drop_edge is identity).

  With a = clip(res_alpha, 0, 1) and Wc = concat_heads(W) [F_in, H*D]:
      out_b = a * (h_b @ Wc) + (1 - a) * h_b = h_b @ (a * Wc + (1 - a) * I)

  so the problem collapses to one [2048,256] @ [256,256] matmul per batch
  sample plus nothing else.  Sharding: data-parallel over B=8 across the 8
  NeuronCores (one sample per core), replicating the fused weight
  M = a*Wc + (1-a)*I.

Numerics: the matmul runs as a bf16 hi/lo split (3 passes, fp32 PSUM
accumulate).  h = hh + hl and M = mh + ml in bf16; bf16 products are exact
in fp32, so  hh@mh + hh@ml + hl@mh  reproduces the fp32 product up to the
dropped hl@ml term (~2^-18 relative).  Measured end-to-end rel err vs the
fp32 reference: ~5e-6.  Each pass streams at 1 cycle/row on the PE (vs 4
for fp32), and input traffic equals fp32 (two bf16 halves = 4 B/elem).

Layout: the TensorEngine contracts along the partition dim for both
operands, so h is fed f-major (pre-transposed on host, cheap).  The kernel
needs zero on-chip transposes and writes the output in natural layout.

Schedule (raw Bass, no Tile framework -> no kernel-tail drain/barrier
butterfly).  DMA count is deliberately small: the HWDGE is a serially
shared resource (~0.6us/DMA) and extra load DMAs starve the store stream,
which otherwise backlogs into a multi-microsecond tail after the PE
finishes (observed in the timeline sim):
  SP   : 8 load DMAs, one per column chunk -- each fills BOTH k-halves of
         a [128, 1024]-bf16 tile ([k0: hh|hl, k1: hh|hl]) via a 3-D access
         pattern.
  ACT  : 1 DMA for m (both k-halves, [mh|ml] each), then 16 store DMAs on
         the second HWDGE ring, final completion wait.
  PE   : two-phase order over the 8 PSUM banks -- all s=0 half-bank groups
         (even token tiles) first, then all s=1 groups -- so copies/stores
         flow at token-tile granularity from early in the kernel instead of
         backlogging into a post-PE tail.  Each group = 6 accumulating
         matmuls (3 bf16 passes x 2 k-chunks).  Before a bank's s=1 group,
         PE waits for the s=0 half's DVE copy (same-bank PE-write +
         DVE-read is a fatal PSUM hazard).
  DVE  : copy each finished half-bank [128,256] into its own SBUF out tile.
"""

from contextlib import ExitStack

import ml_dtypes
import numpy as np

from concourse import bacc, mybir
from concourse.bass_utils import run_bass_kernel_spmd

F32 = mybir.dt.float32
BF16 = mybir.dt.bfloat16

# Problem geometry (fixed by the problem spec).
B, N, F_IN = 8, 2048, 256
H, D = 4, 64
FO = H * D              # 256 output features
N_CORES = 8

P = 128                 # SBUF partitions
KC = F_IN // P          # contraction chunks (2)
TT = N // P             # token tiles (16)
N_CHUNKS = 8            # h column chunks per k-half
N_STORES = 8            # store DMAs
TPC = TT // N_CHUNKS    # token tiles per chunk (2)
CW = TPC * P            # tokens per chunk (256)
TILES_PER_STORE = TT // N_STORES   # 2
STORE_COLS = TILES_PER_STORE * FO  # 512

_NC_CACHE = {}


def _build_nc():
    nc = bacc.Bacc("TRN2", target_bir_lowering=False, debug=False)
    # x row f = k*128 + p; per chunk c, columns [c*2cw, (c+1)*2cw) hold
    # [hh chunk | hl chunk] (bf16 hi/lo split of hT).
    x_d = nc.dram_tensor("x", [F_IN, 2 * N], BF16, kind="ExternalInput")
    # m row f: [mh | ml]
    m_d = nc.dram_tensor("m", [F_IN, 2 * FO], BF16, kind="ExternalInput")
    out_d = nc.dram_tensor("out", [N, FO], F32, kind="ExternalOutput")
    out_t = out_d.rearrange("(t p) d -> t p d", p=P)
    x_v = x_d.rearrange("(k p) w -> p k w", k=KC)
    m_v = m_d.rearrange("(k p) w -> p k w", k=KC)

    # PE/copy/store processing order: phase 0 = even tiles (s=0 halves of
    # banks 0..7), phase 1 = odd tiles.
    order = [2 * g for g in range(8)] + [2 * g + 1 for g in range(8)]

    with ExitStack() as ctx:
        m_sb = ctx.enter_context(
            nc.sbuf_tensor("m01", [P, KC * 2 * FO], BF16)
        )
        h_sb = [
            ctx.enter_context(
                nc.sbuf_tensor(f"h{c}", [P, KC * 2 * CW], BF16)
            )
            for c in range(N_CHUNKS)
        ]
        o_sb = [
            ctx.enter_context(nc.sbuf_tensor(f"o{t}", [P, FO], F32))
            for t in range(TT)
        ]
        ps_banks = [
            ctx.enter_context(nc.psum_tensor(f"ps{g}", [P, 2 * FO], F32))
            for g in range(8)
        ]
        s_m = ctx.enter_context(nc.semaphore("s_m"))
        s_h = [
            ctx.enter_context(nc.semaphore(f"s_h{c}")) for c in range(N_CHUNKS)
        ]
        pe_sem = ctx.enter_context(nc.semaphore("pe_sem"))
        dve_sem = ctx.enter_context(nc.semaphore("dve_sem"))
        out_sem = ctx.enter_context(nc.semaphore("out_sem"))
        block = ctx.enter_context(nc.Block())

        @block.sync
        def _(sync):
            for c in range(N_CHUNKS):
                sync.dma_start(
                    h_sb[c].rearrange("p (k w) -> p k w", k=KC),
                    x_v[:, :, c * 2 * CW : (c + 1) * 2 * CW],
                ).then_inc(s_h[c], 16)

        @block.tensor
        def _(tensor):
            tensor.wait_ge(s_m, 16)
            seen = set()
            for idx, t in enumerate(order):
                g, s = divmod(t, 2)
                c, i = divmod(t, TPC)
                if c not in seen:
                    tensor.wait_ge(s_h[c], 16)
                    seen.add(c)
                if s == 1:
                    # s=0 half of this bank must be copied out before the
                    # bank is written again (same-bank PE-W + DVE-R hazard).
                    tensor.wait_ge(dve_sem, g + 1)
                # (h part, m part) passes; hl@ml dropped (~2^-18).
                passes = [(0, 0), (0, 1), (1, 0)]
                n_mm = len(passes) * KC
                done = 0
                for hp, mp in passes:
                    for k in range(KC):
                        mm = nc.tensor.matmul(
                            ps_banks[g][:, s * FO : (s + 1) * FO],
                            h_sb[c][
                                :,
                                k * 2 * CW + hp * CW + i * P
                                : k * 2 * CW + hp * CW + (i + 1) * P,
                            ],
                            m_sb[:, k * 2 * FO + mp * FO : k * 2 * FO + (mp + 1) * FO],
                            start=(done == 0),
                            stop=(done == n_mm - 1),
                        )
                        done += 1
                        if done == n_mm:
                            mm.then_inc(pe_sem, 1)

        @block.vector
        def _(vector):
            for idx, t in enumerate(order):
                g, s = divmod(t, 2)
                vector.wait_ge(pe_sem, idx + 1)
                nc.vector.tensor_copy(
                    o_sb[t][:], ps_banks[g][:, s * FO : (s + 1) * FO]
                ).then_inc(dve_sem, 1)

        @block.scalar
        def _(scalar):
            scalar.dma_start(
                m_sb.rearrange("p (k w) -> p k w", k=KC), m_v[:]
            ).then_inc(s_m, 16)
            for idx, t in enumerate(order):
                scalar.wait_ge(dve_sem, idx + 1)
                scalar.dma_start(out_t[t], o_sb[t][:]).then_inc(out_sem, 16)
            scalar.wait_ge(out_sem, 16 * TT)

    nc.compile()
    return nc


def _get_nc():
    if "nc" not in _NC_CACHE:
        _NC_CACHE["nc"] = _build_nc()
    return _NC_CACHE["nc"]


def _pack_inputs(h, W, res_alpha):
    bf16 = ml_dtypes.bfloat16
    h = np.asarray(h, dtype=np.float32)
    W = np.asarray(W, dtype=np.float32)

    a = float(np.clip(np.float32(res_alpha), 0.0, 1.0))
    # Concat heads: f_out = head*D + d  ->  Wc[f_in, f_out]
    Wc = np.transpose(W, (1, 0, 2)).reshape(F_IN, FO)
    M = (a * Wc + (1.0 - a) * np.eye(F_IN, dtype=np.float32)).astype(np.float32)
    mh = M.astype(bf16)
    ml = (M - mh.astype(np.float32)).astype(bf16)
    m_packed = np.ascontiguousarray(np.concatenate([mh, ml], axis=1))

    in_maps = []
    for b in range(B):
        hT = np.ascontiguousarray(h[b].T)  # [F_IN, N]
        hh = hT.astype(bf16)
        hl = (hT - hh.astype(np.float32)).astype(bf16)
        x = np.empty((F_IN, 2 * N), dtype=bf16)
        for c in range(N_CHUNKS):
            base = c * 2 * CW
            x[:, base : base + CW] = hh[:, c * CW : (c + 1) * CW]
            x[:, base + CW : base + 2 * CW] = hl[:, c * CW : (c + 1) * CW]
        in_maps.append({"x": x, "m": m_packed})
    return in_maps


def kernel(h, adj, W, res_alpha, **_unused):
    h = np.asarray(h, dtype=np.float32)
    W = np.asarray(W, dtype=np.float32)
    assert h.shape == (B, N, F_IN), h.shape
    assert W.shape == (H, F_IN, D), W.shape

    in_maps = _pack_inputs(h, W, res_alpha)
    res = run_bass_kernel_spmd(_get_nc(), in_maps, list(range(N_CORES)))
    out = np.stack([res.results[b]["out"] for b in range(B)], axis=0)
    return np.ascontiguousarray(out.astype(np.float32))
